# revision 17
# baseline (speedup 1.0000x reference)
"""CIN (Compressed Interaction Network) kernel for Trainium2, 8 NeuronCores.

Problem: x [4096, 39, 16]; 3 CIN layers (size 128 each):
  out_k[b,s,d] = sum_{i,j} x[b,i,d] * prev[b,j,d] * w_k[i*Fk+j, s] + b_k[s]
Output: sum_d concat(out_1, out_2) -> [4096, 256]  (layer 0 output dropped)

Strategy (data-parallel, batch sharded 8 ways, 512 rows/core):
  Activations feature-on-partition (xT [39,bd], outT [128,bd], bd=(b,d)
  b-major, 8192/core). z^T tiles [m-chunk, bd] = in0 (bcast of x rows) *
  in1 (x/out0 rows) via DVE tensor_tensor (bf16), contracted with weight
  chunks on the PE (bf16, fp32 PSUM accum over m).
  Layer 0 uses banded-symmetric pairs: (i, j>=i) with per-band padded run
  length L in {39, 27, 13}; each <=117-row chunk's inputs load as one
  standard DMA per run (partition-broadcast for in0 on the gpsimd queue,
  contiguous row-block copy for in1 on the sync queue), deep-prefetched
  via 4-buffer pools. Out-of-range rows hit zero-padded xTp rows and zero
  weights.
  Layer 1 runs in 4 passes of 2048 cols (4 PSUM banks), leaving banks for
  the layer-2 Gram phase to interleave between passes. Per-i broadcast
  tiles alternate between the sync (HWDGE) and gpsimd (SWDGE) queues with
  6-buffer prefetch.
  Layer 2 never computes out2[b,s,d]: since only sum_d out2 is needed,
    sum_d out2[b,:,d] = w2^T . vec(G2_b) + 16*b2,
    G2_b[i,j] = sum_d x[b,i,d]*out1[b,j,d]  (per-sample Gram, 14x fewer
  FLOPs). G2 via PE-transposing out1T 128-col blocks (8 b's each) times a
  block-diagonal probe built on-chip from a dense [128, 2496] input.
"""
import sys

for p in ("/opt/trn_rl_repo",):
    if p not in sys.path:
        sys.path.insert(0, p)

import contextlib

import numpy as np
import ml_dtypes

import concourse.bass as bass
import concourse.mybir as mybir
import concourse.tile as tile
from concourse import bacc
from concourse.bass_utils import run_bass_kernel_spmd
from concourse.masks import make_identity

F32 = mybir.dt.float32
BF16 = mybir.dt.bfloat16

N_CORES = 8
B, F0, D = 4096, 39, 16
S = 128                      # each CIN layer size
BC = B // N_CORES            # 512 batch rows per core
BD = BC * D                  # 8192
QW = 4096                    # layer-0 half width (8 PSUM banks of 512)
NQ = BD // QW                # 2
PW = 2048                    # layer-1 pass width (4 PSUM banks)
NPASS = BD // PW             # 4
M1 = F0 * S                  # 4992
XTP = 2 * F0                 # padded xT rows (39 real + 39 zero)

# layer-0 banded-symmetric chunks: (i0, L, k) -> k runs i=i0..i0+k-1, each
# padded to L rows t=0..L-1 with pair (i, j=i+t); j>38 or i>38 get zero
# weight and read zero xTp rows.
L0_CHUNKS = [
    (0, 39, 3), (3, 39, 3), (6, 39, 3), (9, 39, 3),          # L=39
    (12, 27, 4), (16, 27, 4), (20, 27, 4), (24, 27, 2),      # L=27
    (26, 13, 9), (35, 13, 9),                                # L=13
]
NC0 = len(L0_CHUNKS)         # 10

GB = 8                       # b's per Gram group (one 128-col block)
NGRP = BC // GB              # 64 Gram groups
NGH = NGRP // 2              # 32 groups per half
GN = GB * F0                 # 312 = Gram matmul free size
FB = 256                     # final matmul b-batch (N=256)


def build_program():
    nc = bacc.Bacc("TRN2", target_bir_lowering=False, debug=False,
                   num_devices=N_CORES)
    xTp = nc.dram_tensor("xTp", [XTP, BD], BF16, kind="ExternalInput").ap()
    w0sb_d = nc.dram_tensor("w0sb", [117, NC0 * S], BF16,
                            kind="ExternalInput").ap()
    w1sb_d = nc.dram_tensor("w1sb", [S, M1], BF16, kind="ExternalInput").ap()
    w2sb_d = nc.dram_tensor("w2sb", [S, M1], BF16, kind="ExternalInput").ap()
    xbdf_d = nc.dram_tensor("xbdf", [128, NGRP * F0], BF16,
                            kind="ExternalInput").ap()
    b0 = nc.dram_tensor("b0", [S, 1], F32, kind="ExternalInput").ap()
    b1 = nc.dram_tensor("b1", [S, 1], F32, kind="ExternalInput").ap()
    b1x = nc.dram_tensor("b1x", [S, 1], F32, kind="ExternalInput").ap()
    b2x = nc.dram_tensor("b2x", [S, 1], F32, kind="ExternalInput").ap()
    out = nc.dram_tensor("out", [BC, 2 * S], F32, kind="ExternalOutput").ap()

    with tile.TileContext(nc) as tc:
        _body(nc, tc, xTp, w0sb_d, w1sb_d, w2sb_d, xbdf_d,
              b0, b1, b1x, b2x, out)
    nc.compile()
    return nc


def _body(nc, tc, xTp, w0sb_d, w1sb_d, w2sb_d, xbdf_d, b0, b1, b1x, b2x, out):
    MUL = mybir.AluOpType.mult
    IDENT = mybir.ActivationFunctionType.Identity
    COPY = mybir.ActivationFunctionType.Copy

    ctx = contextlib.ExitStack()
    with ctx:
        const = ctx.enter_context(tc.tile_pool(name="const", bufs=1))
        acts = ctx.enter_context(tc.tile_pool(name="acts", bufs=1))
        l1p = ctx.enter_context(tc.tile_pool(name="l1", bufs=1))

        # ---- constants / weights (scalar ring: small; sync: w1) ----
        b0t = const.tile([S, 1], F32, tag="b0")
        b1t = const.tile([S, 1], F32, tag="b1")
        b1xt = const.tile([S, 1], F32, tag="b1x")
        b2xt = const.tile([S, 1], F32, tag="b2x")
        nc.scalar.dma_start(out=b0t[:], in_=b0[:])
        nc.scalar.dma_start(out=b1t[:], in_=b1[:])
        nc.scalar.dma_start(out=b1xt[:], in_=b1x[:])
        nc.scalar.dma_start(out=b2xt[:], in_=b2x[:])

        identb = const.tile([128, 128], BF16, tag="identb")
        identf = const.tile([128, 128], F32, tag="identf")
        make_identity(nc, identb[:])
        make_identity(nc, identf[:])

        w0sb = const.tile([117, NC0 * S], BF16, tag="w0")
        nc.scalar.dma_start(out=w0sb[:], in_=w0sb_d[:])
        w1sb = const.tile([S, M1], BF16, tag="w1")
        nc.sync.dma_start(out=w1sb[:], in_=w1sb_d[:])
        w2sb = const.tile([S, M1], BF16, tag="w2")
        nc.scalar.dma_start(out=w2sb[:], in_=w2sb_d[:])
        xbdf = const.tile([128, NGRP * F0], BF16, tag="xbdf")
        nc.scalar.dma_start(out=xbdf[:], in_=xbdf_d[:])

        out0T = acts.tile([S, BD], BF16, tag="out0T")
        out1T = acts.tile([S, BD], BF16, tag="out1T")
        outF1 = acts.tile([S, BC], F32, tag="outF1")
        outF2 = acts.tile([S, BC], F32, tag="outF2")

        # ================= shared PSUM pool (8 bank slots) =============
        # every tile is one 2KB bank; gram/final tiles are sliced/bitcast
        # views of the same slots so phases can interleave without
        # exceeding the 8-bank PSUM.
        psum_ctx = contextlib.ExitStack()
        ps = psum_ctx.enter_context(
            tc.tile_pool(name="ps", bufs=1, space="PSUM"))

        def pbank(tag):
            return ps.tile([S, 512], F32, tag=tag, name=tag, bufs=1)

        l0_ctx = contextlib.ExitStack()
        l0p = l0_ctx.enter_context(tc.tile_pool(name="l0", bufs=1))

        def l0_q(q):
            qsl = slice(q * QW, (q + 1) * QW)
            accs = [pbank(f"a{g}") for g in range(QW // 512)]
            for c, (i0, L, k) in enumerate(L0_CHUNKS):
                n = k * L
                in0z = l0p.tile([117, QW], BF16, tag="in0z", bufs=4,
                                name="in0z")
                in1z = l0p.tile([117, QW], BF16, tag="in1z", bufs=4,
                                name="in1z")
                for u in range(k):
                    usl = slice(u * L, (u + 1) * L)
                    nc.gpsimd.dma_start(
                        out=in0z[usl, :],
                        in_=xTp[i0 + u:i0 + u + 1, qsl].partition_broadcast(L))
                    nc.sync.dma_start(out=in1z[usl, :],
                                      in_=xTp[i0 + u:i0 + u + L, qsl])
                z = l0p.tile([117, QW], BF16, tag="z", bufs=4, name="z")
                nc.vector.tensor_tensor(out=z[:n, :], in0=in1z[:n, :],
                                        in1=in0z[:n, :], op=MUL)
                lhsT = w0sb[:n, S * c:S * (c + 1)]
                for g in range(QW // 512):
                    nc.tensor.matmul(accs[g][:], lhsT,
                                     z[:n, 512 * g:512 * (g + 1)],
                                     start=(c == 0), stop=(c == NC0 - 1))
            for g in range(QW // 512):
                nc.scalar.activation(
                    out0T[:, q * QW + 512 * g:q * QW + 512 * (g + 1)],
                    accs[g][:], IDENT, bias=b0t[:], scale=1.0)

        def l1_pass(p, tags):
            psl = slice(p * PW, (p + 1) * PW)
            accs = [pbank(tags[g]) for g in range(PW // 512)]
            for i in range(F0):
                bc = l1p.tile([128, PW], BF16, tag="bcb", bufs=6, name="bcb")
                eng = nc.sync if i % 2 == 0 else nc.gpsimd
                eng.dma_start(out=bc[:],
                              in_=xTp[i:i + 1, psl].partition_broadcast(128))
                z = l1p.tile([128, PW], BF16, tag="z1", bufs=4, name="z1")
                nc.vector.tensor_tensor(out=z[:], in0=out0T[:, psl],
                                        in1=bc[:], op=MUL)
                lhsT = w1sb[:, S * i:S * (i + 1)]
                for g in range(PW // 512):
                    nc.tensor.matmul(accs[g][:], lhsT,
                                     z[:, 512 * g:512 * (g + 1)],
                                     start=(i == 0), stop=(i == F0 - 1))
            for g in range(PW // 512):
                gco = p * PW + 512 * g
                nc.scalar.activation(out1T[:, gco:gco + 512], accs[g][:],
                                     IDENT, bias=b1t[:], scale=1.0)
                nc.vector.tensor_reduce(
                    out=outF1[:, gco // D:gco // D + 512 // D],
                    in_=accs[g][:].rearrange("p (b d) -> p b d", d=D),
                    axis=mybir.AxisListType.X, op=mybir.AluOpType.add)

        def gram_half(h, g2s, tags):
            for gl in range(NGH):
                gg = NGH * h + gl
                tps = pbank(tags[gl % 2]).bitcast(BF16)[:, 0:128]
                nc.tensor.transpose(tps, out1T[:, 128 * gg:128 * (gg + 1)],
                                    identb[:])
                o1t = gram.tile([128, 128], BF16, tag="o1t", bufs=2,
                                name="o1t")
                nc.scalar.activation(o1t[:], tps, COPY)
                g2p = pbank(tags[2 + gl % 2])[:, 0:GN]
                nc.tensor.matmul(g2p, o1t[:],
                                 xbdh[:, GN * gl:GN * (gl + 1)],
                                 start=True, stop=True)
                dst = (g2s[:].rearrange("p (i b) -> p i b", i=F0)
                       [:, :, GB * gl:GB * (gl + 1)])
                nc.scalar.activation(
                    dst, g2p.rearrange("p (k i) -> p i k", k=GB), COPY)

        # interleaved phase order: L0q1 runs after L1p0/p1 so its DMA
        # latency hides under their compute.
        l0_q(0)
        l1_pass(0, ("a0", "a1", "a2", "a3"))
        l1_pass(1, ("a4", "a5", "a6", "a7"))
        l0_q(1)
        l0_ctx.close()

        # gram tiles (pool opens after l0 frees its SBUF; xbdh rebuilt
        # between halves)
        gram = ctx.enter_context(tc.tile_pool(name="gram", bufs=1))
        xbdh = gram.tile([128, NGH * GN], BF16, tag="xbdh")
        g2s0 = gram.tile([S, NGH * GN], BF16, tag="g2s0")
        g2s1 = gram.tile([S, NGH * GN], BF16, tag="g2s1")
        nc.vector.memset(xbdh[:], 0)
        _xbd_fill(nc, xbdh, xbdf, 0)

        gram_half(0, g2s0, ("a0", "a1", "a2", "a3"))
        l1_pass(2, ("a4", "a5", "a6", "a7"))
        l1_pass(3, ("a0", "a1", "a2", "a3"))
        nc.vector.memset(xbdh[:], 0)
        _xbd_fill(nc, xbdh, xbdf, 1)
        gram_half(1, g2s1, ("a4", "a5", "a6", "a7"))

        # ================= final contraction =================
        # +16*b1 on outF1 (sum_d bias)
        nc.vector.tensor_scalar_add(outF1[:], outF1[:], b1xt[:])

        for sgi, g2s in enumerate((g2s0, g2s1)):
            facc = pbank(f"a{sgi}")[:, 0:FB]
            for i in range(F0):
                nc.tensor.matmul(facc, w2sb[:, S * i:S * (i + 1)],
                                 g2s[:, FB * i:FB * (i + 1)],
                                 start=(i == 0), stop=(i == F0 - 1))
            nc.scalar.activation(outF2[:, FB * sgi:FB * (sgi + 1)], facc,
                                 IDENT, bias=b2xt[:], scale=1.0)

        # ================= output assembly: out[b, s] =================
        for t in range(BC // 128):
            csl = slice(128 * t, 128 * (t + 1))
            otile = gram.tile([128, 2 * S], F32, tag="outsb", bufs=2,
                              name="outsb")
            p1 = pbank(f"a{2 + t % 2}")[:, 0:128]
            nc.tensor.transpose(p1, outF1[:, csl], identf[:])
            nc.vector.tensor_copy(otile[:, 0:S], p1)
            p2 = pbank(f"a{4 + t % 2}")[:, 0:128]
            nc.tensor.transpose(p2, outF2[:, csl], identf[:])
            nc.vector.tensor_copy(otile[:, S:2 * S], p2)
            nc.sync.dma_start(out=out[csl, :], in_=otile[:])
        psum_ctx.close()


def _xbd_fill(nc, xbdh, xbdf, h):
    """Scatter dense probe content into the block-diagonal layout.
    Within-group columns are (kk, i) so each k-slice is 39 contiguous
    elements: xbdh[16k+d, (gl, kk, i)] = xbdf[16k+d, (32h+gl, i)] iff
    kk == k."""
    for k in range(GB):
        rsl = slice(16 * k, 16 * (k + 1))
        dst = (xbdh[rsl, :].rearrange("p (g kk i) -> p g kk i", g=NGH, kk=GB)
               [:, :, k, :])
        src = (xbdf[rsl, NGH * F0 * h:NGH * F0 * (h + 1)]
               .rearrange("p (g i) -> p g i", g=NGH))
        nc.gpsimd.dma_start(out=dst, in_=src)


_PROGRAM_CACHE = {}


def _get_program():
    if "nc" not in _PROGRAM_CACHE:
        _PROGRAM_CACHE["nc"] = build_program()
    return _PROGRAM_CACHE["nc"]


def host_prep(x, w0, b0, w1, b1, w2, b2):
    bf = ml_dtypes.bfloat16
    x = np.asarray(x, dtype=np.float32)

    # layer-0 banded-symmetric weight packing
    w0f = np.asarray(w0, np.float32).reshape(F0, F0, S)
    w0sym = w0f + np.transpose(w0f, (1, 0, 2))
    w0sb = np.zeros((117, NC0 * S), np.float32)
    for c, (i0, L, k) in enumerate(L0_CHUNKS):
        for u in range(k):
            i = i0 + u
            for t in range(L):
                j = i + t
                if i < F0 and j < F0:
                    w0sb[u * L + t, S * c:S * (c + 1)] = (
                        w0f[i, j] if t == 0 else w0sym[i, j])
    w0sb = np.ascontiguousarray(w0sb.astype(bf))

    w1f = np.asarray(w1, np.float32).reshape(F0, S, S)
    w1sb = np.ascontiguousarray(
        w1f.transpose(1, 0, 2).reshape(S, M1).astype(bf))
    w2f = np.asarray(w2, np.float32).reshape(F0, S, S)
    w2sb = np.ascontiguousarray(
        w2f.transpose(1, 0, 2).reshape(S, M1).astype(bf))

    b0v = np.ascontiguousarray(np.asarray(b0, np.float32).reshape(S, 1))
    b1v = np.ascontiguousarray(np.asarray(b1, np.float32).reshape(S, 1))
    b1x = np.ascontiguousarray(D * np.asarray(b1, np.float32).reshape(S, 1))
    b2x = np.ascontiguousarray(D * np.asarray(b2, np.float32).reshape(S, 1))

    in_maps = []
    for cid in range(N_CORES):
        xs = x[BC * cid:BC * (cid + 1)]                  # [512, 39, 16]
        xTp = np.zeros((XTP, BD), np.float32)
        xTp[:F0] = xs.transpose(1, 0, 2).reshape(F0, BD)
        xTp = np.ascontiguousarray(xTp.astype(bf))
        # dense gram probe content: row 16k+d, col (g, i) = x[8g+k, i, d]
        blk = xs.reshape(NGRP, GB, F0, D)                # [g, k, i, d]
        xbdf = np.ascontiguousarray(
            blk.transpose(1, 3, 0, 2)                    # [k, d, g, i]
            .reshape(128, NGRP * F0).astype(bf))
        in_maps.append({"xTp": xTp, "w0sb": w0sb, "w1sb": w1sb,
                        "w2sb": w2sb, "xbdf": xbdf,
                        "b0": b0v, "b1": b1v, "b1x": b1x, "b2x": b2x})
    return in_maps


def kernel(x, w0, b0, w1, b1, w2, b2):
    in_maps = host_prep(x, w0, b0, w1, b1, w2, b2)
    nc = _get_program()
    res = run_bass_kernel_spmd(nc, in_maps, core_ids=list(range(N_CORES)),
                               trace=False)
    return np.concatenate([r["out"] for r in res.results], axis=0)
